# revision 26
# baseline (speedup 1.0000x reference)
"""Trainium2 Bass kernel for cubic (Keys) interpolation of vertices in a 3D volume.

bf16 shingle + slab-bucketed dma_gather + host-precomputed weight table
+ 2x-mode DVE multiply/tree pipeline.

Sharding: vertices are sorted by shingle row (host side) and split into 8
equal rank-ranges, one per NeuronCore. The volume is stored per core as a
bf16 shingle
    S[x, y, z, c, dx, dy] = vol[c, x+dx, y+dy, z]     (rows of 128 bf16)
with only the row-ranges that core's vertices touch, repacked into NSLAB
fixed-stride slabs so every gather index fits int16 (dma_gather's index
dtype). Rows (x,y,z)..(x,y,z+3) -- one 1KB run -- hold a vertex's whole
4x4x4x8 neighborhood in payload order [k(z):4, c:8, i(dx):4, j(dy):4];
channel sits mid-payload so the weight broadcast keeps innermost step=1 and
the large DVE ops run in 2x_1P bf16 mode.

Per core: NSLAB dma_gather calls (ONE SWDGE instruction each, spread over 4
SWDGE queues) fetch GROUP=896 vertex neighborhoods per call, landing index
j at (partition j%128, column j//128). Groups are padded to exactly 896
with fake vertices so the call structure is SPMD-uniform; fake outputs are
dropped at reassembly. The 64 Keys-kernel weights per vertex (wz*wx*wy,
separable outer product) are precomputed on the host in fp32, cast to bf16,
and uploaded as a [128, M*64] table -- they are O(V) metadata derived from
the same clip/floor as the gather indices. The device does the heavy part:
gather 1KB/vertex, multiply by the broadcast weights (DVE 2x), tree-reduce
k, i, j (2 ops each; the last j-add writes the compact f32 output), DMA out.
"""

import numpy as np
import ml_dtypes

import concourse.bass as bass
import concourse.tile as tile
from concourse import bacc, mybir
from concourse.bass_utils import run_bass_kernel_spmd

X, Y, Z, C = 112, 224, 160, 8
P = 128
NCORES = 8
V = 150000
VCORE = V // NCORES          # 18750
GROUP = 896                  # indices per dma_gather call (7 columns)
GCOLS = GROUP // P           # 7
SPAN_CAP = 32700             # max row span within one slab (int16 margin)
BSTRIDE = 32772              # rows per slab in the repacked shingle
ES = 512                     # elems gathered per index (4 rows x 128)
STEP = 128                   # elems per row
TILE_SLABS = 4               # slabs processed per DVE batch
GBUFS = 5
MAGIC = 12582912.0           # 1.5 * 2**23 fp32 round-to-int magic

BF16 = mybir.dt.bfloat16
F32 = mybir.dt.float32
I16 = mybir.dt.int16
ALU = mybir.AluOpType
ACT = mybir.ActivationFunctionType

_CACHE = {}


# --------------------------------------------------------------------------
# device program
# --------------------------------------------------------------------------

def _build_program(nslab):
    m = nslab * GCOLS  # total slot-columns per partition
    srows = nslab * BSTRIDE
    nc = bacc.Bacc("TRN2", target_bir_lowering=False, debug=False,
                   num_devices=NCORES, num_swdge_queues=4)
    s_in = nc.dram_tensor("shingle", [srows, P], BF16, kind="ExternalInput").ap()
    w_in = nc.dram_tensor("wtab", [P, m * 64], BF16, kind="ExternalInput").ap()
    idx_in = nc.dram_tensor("idx", [P, nslab * (GROUP // 16)], I16,
                            kind="ExternalInput").ap()
    out_ext = nc.dram_tensor("out", [P, m * C], F32, kind="ExternalOutput").ap()

    with tile.TileContext(nc) as tc:
        _emit(tc, nslab, out_ext, w_in, idx_in, s_in)
    nc.compile()
    return nc


def _emit(tc, nslab, out_ext, w_in, idx_in, s_in):
    nc = tc.nc
    vec = nc.vector
    m = nslab * GCOLS

    with (
        tc.tile_pool(name="keep", bufs=1) as keep,
        tc.tile_pool(name="gpool", bufs=GBUFS) as gpool,
        tc.tile_pool(name="opool", bufs=2) as opool,
    ):
        Wall = keep.tile([P, m * 64], BF16)
        idx = keep.tile([P, nslab * (GROUP // 16)], I16)

        nc.sync.dma_start(out=idx[:], in_=idx_in)
        nc.sync.dma_start(out=Wall[:], in_=w_in)

        # ramp-up batch schedule: small first batches so the DVE starts
        # as soon as the first slab lands, then steady TILE_SLABS batches
        sizes = [1, 2]
        left = nslab - 3
        while left > 0:
            t = min(TILE_SLABS, left)
            sizes.append(t)
            left -= t
        batches = []
        b0 = 0
        for t in sizes:
            batches.append((b0, b0 + t))
            b0 += t

        for (b0, b1) in batches:
            nb = b1 - b0
            ns = nb * GCOLS          # slots this batch
            s0 = b0 * GCOLS
            G = gpool.tile([P, TILE_SLABS * GCOLS * ES], BF16, tag="G")
            ot = opool.tile([P, TILE_SLABS * GCOLS * C], F32, tag="ot")

            # one dma_gather per slab: GROUP indices x 1KB runs
            for b in range(b0, b1):
                src_win = bass.AP(s_in.tensor, b * BSTRIDE * STEP,
                                  [[STEP, SPAN_CAP + 8], [1, ES]])
                gv = G[:, (b - b0) * GCOLS * ES:(b - b0 + 1) * GCOLS * ES] \
                    .rearrange("p (t e) -> p t e", e=ES)
                nc.gpsimd.dma_gather(
                    out_ap=gv, in_ap=src_win,
                    idxs_ap=idx[:, b * (GROUP // 16):(b + 1) * (GROUP // 16)],
                    num_idxs=GROUP, num_idxs_reg=GROUP,
                    elem_size=ES, elem_step=STEP, queue_num=b % 4)

            # G *= W  (payload [k, c, i, j]; c broadcast mid-dim -> 2x).
            # One op per k keeps each AP at 3 free dims (4-dim + broadcast
            # measured ~20% over formula rate).
            Gv = G[:, :ns * ES].rearrange("p (s k c e) -> p s k c e",
                                          s=ns, k=4, c=8, e=16)
            Wv = Wall[:, s0 * 64:(s0 + ns) * 64] \
                .rearrange("p (s k e) -> p s k e", s=ns, k=4)
            for kk in range(4):
                vec.tensor_tensor(
                    out=Gv[:, :, kk], in0=Gv[:, :, kk],
                    in1=Wv[:, :, kk].unsqueeze(2).to_broadcast([P, ns, 8, 16]),
                    op=ALU.mult)

            # k-tree (stride 128), 2 ops
            Gk = G[:, :ns * ES].rearrange("p (s k r) -> p s k r",
                                          s=ns, k=4, r=128)
            vec.tensor_tensor(out=Gk[:, :, 0:2], in0=Gk[:, :, 0:2],
                              in1=Gk[:, :, 2:4], op=ALU.add)
            vec.tensor_tensor(out=Gk[:, :, 0:1], in0=Gk[:, :, 0:1],
                              in1=Gk[:, :, 1:2], op=ALU.add)
            # i-tree, 2 ops
            Gi = G[:, :ns * ES].rearrange("p (s k c i j) -> p s k c i j",
                                          s=ns, k=4, c=8, i=4, j=4)[:, :, 0]
            vec.tensor_tensor(out=Gi[:, :, :, 0:2], in0=Gi[:, :, :, 0:2],
                              in1=Gi[:, :, :, 2:4], op=ALU.add)
            vec.tensor_tensor(out=Gi[:, :, :, 0:1], in0=Gi[:, :, :, 0:1],
                              in1=Gi[:, :, :, 1:2], op=ALU.add)
            # j-tree, 2 ops
            Gj = Gi[:, :, :, 0]
            vec.tensor_tensor(out=Gj[:, :, :, 0:2], in0=Gj[:, :, :, 0:2],
                              in1=Gj[:, :, :, 2:4], op=ALU.add)
            # final j-add writes the compact f32 output view directly
            vec.tensor_tensor(
                out=ot[:, :ns * C].rearrange("p (s c) -> p s c", c=C)
                    .unsqueeze(3),
                in0=Gj[:, :, :, 0:1], in1=Gj[:, :, :, 1:2], op=ALU.add)
            nc.sync.dma_start(out=out_ext[:, s0 * C:(s0 + ns) * C],
                              in_=ot[:, :ns * C])


def _get_program(nslab):
    key = ("nc", nslab)
    if key not in _CACHE:
        _CACHE[key] = _build_program(nslab)
    return _CACHE[key]


# --------------------------------------------------------------------------
# host-side preparation
# --------------------------------------------------------------------------

def _f32_to_bf16_bits(a):
    b = a.view(np.uint32)
    rounded = b + 0x7FFF + ((b >> 16) & 1)
    return (rounded >> 16).astype(np.uint16)


def _build_shingle_u16(vol):
    """S[x, y, z, c, dx, dy] = vol[c, x+dx, y+dy, z], flat [NROW, 128] u16."""
    v = np.ascontiguousarray(np.asarray(vol[0], dtype=np.float32))  # (C,X,Y,Z)
    vb = _f32_to_bf16_bits(v)
    vt = np.ascontiguousarray(vb.transpose(1, 2, 3, 0))             # (X,Y,Z,C)
    S = np.zeros((X, Y, Z, C, 4, 4), np.uint16)
    for dx in range(4):
        for dy in range(4):
            S[:X - dx, :Y - dy, :, :, dx, dy] = vt[dx:, dy:, :, :]
    return S.reshape(X * Y * Z, 128)


def _clip_floor(vert):
    """Exact replica of fp32 clip + magic-floor; returns (vc, fl)."""
    v = np.asarray(vert[0], dtype=np.float32)
    vc = np.empty_like(v)
    for d, dim in enumerate((X, Y, Z)):
        vc[:, d] = np.clip(v[:, d], np.float32(1.0 + 1e-5),
                           np.float32(dim - 2 - 1e-5))
    mg = np.float32(MAGIC)
    fl = ((vc - np.float32(0.5)) + mg) - mg
    return vc, fl


def _host_rows(fl):
    fli = fl.astype(np.int64)
    return ((fli[:, 0] - 1) * Y + (fli[:, 1] - 1)) * Z + (fli[:, 2] - 1)


def _host_weights(vc, fl):
    """Keys cubic weights -> [V, 4(k/z), 4(i/x), 4(j/y)] bf16 bits (u16)."""
    u = (vc - fl).astype(np.float32)
    u2 = u * u
    u3 = u2 * u
    w0 = -u3 + 2 * u2 - u
    w1 = 3 * u3 - (5 * u2 - 2)
    w2 = -3 * u3 + (4 * u2 + u)
    w3 = u3 - u2
    w = np.stack([w0, w1, w2, w3], axis=2)   # [V, d, i], raw 2x weights
    w[:, 2, :] *= np.float32(0.125)          # fold 1/8 into z
    wx, wy, wz = w[:, 0], w[:, 1], w[:, 2]
    W = (wz[:, :, None, None] * wx[:, None, :, None] *
         wy[:, None, None, :]).astype(np.float32)      # [V, k, i, j]
    return _f32_to_bf16_bits(W.reshape(len(vc), 64))


def _prepare(vert, vol):
    vc, fl = _clip_floor(vert)
    rows = _host_rows(fl)
    wbits = _host_weights(vc, fl)                # [V, 64] u16
    order = np.argsort(rows, kind="stable")
    Sfull = _build_shingle_u16(vol)

    cores = []
    nslab_needed = 0
    for c in range(NCORES):
        ids = order[c * VCORE:(c + 1) * VCORE]
        r = rows[ids]
        groups = []
        i = 0
        n = len(ids)
        while i < n:
            jmax = min(i + GROUP, n)
            j = int(np.searchsorted(r, r[i] + SPAN_CAP, side="right"))
            j = min(j, jmax)
            groups.append((i, j))
            i = j
        cores.append((ids, r, groups))
        nslab_needed = max(nslab_needed, len(groups))
    nslab = nslab_needed
    m = nslab * GCOLS
    srows = nslab * BSTRIDE

    in_maps = []
    perms = []
    for c in range(NCORES):
        ids, r, groups = cores[c]
        sh = np.zeros((srows, P), np.uint16)
        wtab = np.zeros((m * P, 64), np.uint16)
        idx16 = np.zeros((nslab, 16, GROUP // 16), np.int16)
        flat_pos = np.empty(len(ids), np.int64)
        for b in range(nslab):
            if b >= len(groups):
                continue
            i, j = groups[b]
            cnt = j - i
            r0 = int(r[i])
            r1 = int(r[j - 1])
            nrow = min(r1 - r0 + 4, srows - b * BSTRIDE)
            sh[b * BSTRIDE:b * BSTRIDE + nrow] = Sfull[r0:r0 + nrow]
            full_rel = np.zeros(GROUP, np.int16)
            full_rel[:cnt] = (r[i:j] - r0).astype(np.int16)
            idx16[b] = full_rel.reshape(GROUP // 16, 16).T
            jj = np.arange(cnt)
            part = jj % P
            col = b * GCOLS + jj // P
            flat_pos[i:i + cnt] = part * m + col
            wtab[part * m + col] = wbits[ids[i:j]]
        in_maps.append({
            "shingle": sh.view(ml_dtypes.bfloat16),
            "wtab": np.ascontiguousarray(
                wtab.reshape(P, m * 64)).view(ml_dtypes.bfloat16),
            "idx": np.ascontiguousarray(
                np.tile(idx16.transpose(1, 0, 2).reshape(16, -1), (8, 1))),
        })
        perms.append((ids, flat_pos))
    return nslab, in_maps, perms


def run_cores(vert, vol, trace=False, n_cores=NCORES, **kwargs):
    nslab, in_maps, perms = _prepare(vert, vol)
    nc = _get_program(nslab)
    res = run_bass_kernel_spmd(nc, in_maps, list(range(n_cores)),
                               trace=trace, **kwargs)
    m = nslab * GCOLS
    full = np.zeros((1, V, C), np.float32)
    for c in range(n_cores):
        out = np.asarray(res.results[c]["out"]).reshape(P * m, C)
        ids, flat_pos = perms[c]
        full[0, ids] = out[flat_pos]
    return full, res


def kernel(vert, vol):
    full, _ = run_cores(vert, vol, trace=False)
    return full


# revision 28
# speedup vs baseline: 1.0500x; 1.0500x over previous
"""Trainium2 Bass kernel for cubic (Keys) interpolation of vertices in a 3D volume.

bf16 shingle + slab-bucketed dma_gather + host-precomputed weight table
+ 2x-mode DVE multiply/tree pipeline.

Sharding: vertices are sorted by shingle row (host side) and split into 8
equal rank-ranges, one per NeuronCore. The volume is stored per core as a
bf16 shingle
    S[x, y, z, c, dx, dy] = vol[c, x+dx, y+dy, z]     (rows of 128 bf16)
with only the row-ranges that core's vertices touch, repacked into NSLAB
fixed-stride slabs so every gather index fits int16 (dma_gather's index
dtype). Rows (x,y,z)..(x,y,z+3) -- one 1KB run -- hold a vertex's whole
4x4x4x8 neighborhood in payload order [k(z):4, c:8, i(dx):4, j(dy):4];
channel sits mid-payload so the weight broadcast keeps innermost step=1 and
the large DVE ops run in 2x_1P bf16 mode.

Per core: NSLAB dma_gather calls (ONE SWDGE instruction each, spread over 4
SWDGE queues) fetch GROUP=896 vertex neighborhoods per call, landing index
j at (partition j%128, column j//128). Groups are padded to exactly 896
with fake vertices so the call structure is SPMD-uniform; fake outputs are
dropped at reassembly. The 64 Keys-kernel weights per vertex (wz*wx*wy,
separable outer product) are precomputed on the host in fp32, cast to bf16,
and uploaded as a [128, M*64] table -- they are O(V) metadata derived from
the same clip/floor as the gather indices. The device does the heavy part:
gather 1KB/vertex, multiply by the broadcast weights (DVE 2x), tree-reduce
k, i, j (2 ops each; the last j-add writes the compact f32 output), DMA out.
"""

import numpy as np
import ml_dtypes

import concourse.bass as bass
import concourse.tile as tile
from concourse import bacc, mybir
from concourse.bass_utils import run_bass_kernel_spmd

X, Y, Z, C = 112, 224, 160, 8
P = 128
NCORES = 8
V = 150000
VCORE = V // NCORES          # 18750
GROUP = 896                  # indices per dma_gather call (7 columns)
GCOLS = GROUP // P           # 7
SPAN_CAP = 32700             # max row span within one slab (int16 margin)
BSTRIDE = 32772              # rows per slab in the repacked shingle
ES = 512                     # elems gathered per index (4 rows x 128)
STEP = 128                   # elems per row
TILE_SLABS = 4               # slabs processed per DVE batch
GBUFS = 5
MAGIC = 12582912.0           # 1.5 * 2**23 fp32 round-to-int magic

BF16 = mybir.dt.bfloat16
F32 = mybir.dt.float32
I16 = mybir.dt.int16
ALU = mybir.AluOpType
ACT = mybir.ActivationFunctionType

_CACHE = {}


# --------------------------------------------------------------------------
# device program
# --------------------------------------------------------------------------

def _build_program(nslab):
    m = nslab * GCOLS  # total slot-columns per partition
    srows = nslab * BSTRIDE
    nc = bacc.Bacc("TRN2", target_bir_lowering=False, debug=False,
                   num_devices=NCORES, num_swdge_queues=4)
    s_in = nc.dram_tensor("shingle", [srows, P], BF16, kind="ExternalInput").ap()
    w_in = nc.dram_tensor("wtab", [P, m * 64], BF16, kind="ExternalInput").ap()
    idx_in = nc.dram_tensor("idx", [P, nslab * (GROUP // 16)], I16,
                            kind="ExternalInput").ap()
    out_ext = nc.dram_tensor("out", [P, m * C], F32, kind="ExternalOutput").ap()

    with tile.TileContext(nc) as tc:
        _emit(tc, nslab, out_ext, w_in, idx_in, s_in)
    nc.compile()
    return nc


def _emit(tc, nslab, out_ext, w_in, idx_in, s_in):
    nc = tc.nc
    vec = nc.vector
    m = nslab * GCOLS

    with (
        tc.tile_pool(name="keep", bufs=1) as keep,
        tc.tile_pool(name="gpool", bufs=GBUFS) as gpool,
        tc.tile_pool(name="opool", bufs=2) as opool,
    ):
        Wall = keep.tile([P, m * 64], BF16)
        idx = keep.tile([P, nslab * (GROUP // 16)], I16)

        nc.sync.dma_start(out=idx[:], in_=idx_in)
        nc.sync.dma_start(out=Wall[:], in_=w_in)

        # ramp-up batch schedule: small first batches so the DVE starts
        # as soon as the first slab lands, then steady TILE_SLABS batches
        sizes = [1, 2]
        left = nslab - 3
        while left > 0:
            t = min(TILE_SLABS, left)
            sizes.append(t)
            left -= t
        batches = []
        b0 = 0
        for t in sizes:
            batches.append((b0, b0 + t))
            b0 += t

        late_b0 = batches[-2][0]
        for (b0, b1) in batches:
            nb = b1 - b0
            ns = nb * GCOLS          # slots this batch
            s0 = b0 * GCOLS
            # the Pool engine is idle after its last gather while the DVE
            # finishes; give it the small i/j tree ops of the late batches
            tre = nc.gpsimd if b0 >= late_b0 else vec
            G = gpool.tile([P, TILE_SLABS * GCOLS * ES], BF16, tag="G")
            ot = opool.tile([P, TILE_SLABS * GCOLS * C], F32, tag="ot")

            # one dma_gather per slab: GROUP indices x 1KB runs
            for b in range(b0, b1):
                src_win = bass.AP(s_in.tensor, b * BSTRIDE * STEP,
                                  [[STEP, SPAN_CAP + 8], [1, ES]])
                gv = G[:, (b - b0) * GCOLS * ES:(b - b0 + 1) * GCOLS * ES] \
                    .rearrange("p (t e) -> p t e", e=ES)
                nc.gpsimd.dma_gather(
                    out_ap=gv, in_ap=src_win,
                    idxs_ap=idx[:, b * (GROUP // 16):(b + 1) * (GROUP // 16)],
                    num_idxs=GROUP, num_idxs_reg=GROUP,
                    elem_size=ES, elem_step=STEP, queue_num=b % 4)

            # G *= W  (payload [k, c, i, j]; c broadcast mid-dim -> 2x)
            Gv = G[:, :ns * ES].rearrange("p (s k c e) -> p s k c e",
                                          s=ns, k=4, c=8, e=16)
            vec.tensor_tensor(
                out=Gv, in0=Gv,
                in1=Wall[:, s0 * 64:(s0 + ns) * 64]
                    .rearrange("p (s k e) -> p s k e", s=ns, k=4)
                    .unsqueeze(3).to_broadcast([P, ns, 4, 8, 16]),
                op=ALU.mult)

            # k-tree (stride 128), 2 ops
            Gk = G[:, :ns * ES].rearrange("p (s k r) -> p s k r",
                                          s=ns, k=4, r=128)
            vec.tensor_tensor(out=Gk[:, :, 0:2], in0=Gk[:, :, 0:2],
                              in1=Gk[:, :, 2:4], op=ALU.add)
            vec.tensor_tensor(out=Gk[:, :, 0:1], in0=Gk[:, :, 0:1],
                              in1=Gk[:, :, 1:2], op=ALU.add)
            # i-tree, 2 ops
            Gi = G[:, :ns * ES].rearrange("p (s k c i j) -> p s k c i j",
                                          s=ns, k=4, c=8, i=4, j=4)[:, :, 0]
            tre.tensor_tensor(out=Gi[:, :, :, 0:2], in0=Gi[:, :, :, 0:2],
                              in1=Gi[:, :, :, 2:4], op=ALU.add)
            tre.tensor_tensor(out=Gi[:, :, :, 0:1], in0=Gi[:, :, :, 0:1],
                              in1=Gi[:, :, :, 1:2], op=ALU.add)
            # j-tree, 2 ops
            Gj = Gi[:, :, :, 0]
            tre.tensor_tensor(out=Gj[:, :, :, 0:2], in0=Gj[:, :, :, 0:2],
                              in1=Gj[:, :, :, 2:4], op=ALU.add)
            # final j-add writes the compact f32 output view directly
            vec.tensor_tensor(
                out=ot[:, :ns * C].rearrange("p (s c) -> p s c", c=C)
                    .unsqueeze(3),
                in0=Gj[:, :, :, 0:1], in1=Gj[:, :, :, 1:2], op=ALU.add)
            nc.sync.dma_start(out=out_ext[:, s0 * C:(s0 + ns) * C],
                              in_=ot[:, :ns * C])


def _get_program(nslab):
    key = ("nc", nslab)
    if key not in _CACHE:
        _CACHE[key] = _build_program(nslab)
    return _CACHE[key]


# --------------------------------------------------------------------------
# host-side preparation
# --------------------------------------------------------------------------

def _f32_to_bf16_bits(a):
    b = a.view(np.uint32)
    rounded = b + 0x7FFF + ((b >> 16) & 1)
    return (rounded >> 16).astype(np.uint16)


def _build_shingle_u16(vol):
    """S[x, y, z, c, dx, dy] = vol[c, x+dx, y+dy, z], flat [NROW, 128] u16."""
    v = np.ascontiguousarray(np.asarray(vol[0], dtype=np.float32))  # (C,X,Y,Z)
    vb = _f32_to_bf16_bits(v)
    vt = np.ascontiguousarray(vb.transpose(1, 2, 3, 0))             # (X,Y,Z,C)
    S = np.zeros((X, Y, Z, C, 4, 4), np.uint16)
    for dx in range(4):
        for dy in range(4):
            S[:X - dx, :Y - dy, :, :, dx, dy] = vt[dx:, dy:, :, :]
    return S.reshape(X * Y * Z, 128)


def _clip_floor(vert):
    """Exact replica of fp32 clip + magic-floor; returns (vc, fl)."""
    v = np.asarray(vert[0], dtype=np.float32)
    vc = np.empty_like(v)
    for d, dim in enumerate((X, Y, Z)):
        vc[:, d] = np.clip(v[:, d], np.float32(1.0 + 1e-5),
                           np.float32(dim - 2 - 1e-5))
    mg = np.float32(MAGIC)
    fl = ((vc - np.float32(0.5)) + mg) - mg
    return vc, fl


def _host_rows(fl):
    fli = fl.astype(np.int64)
    return ((fli[:, 0] - 1) * Y + (fli[:, 1] - 1)) * Z + (fli[:, 2] - 1)


def _host_weights(vc, fl):
    """Keys cubic weights -> [V, 4(k/z), 4(i/x), 4(j/y)] bf16 bits (u16)."""
    u = (vc - fl).astype(np.float32)
    u2 = u * u
    u3 = u2 * u
    w0 = -u3 + 2 * u2 - u
    w1 = 3 * u3 - (5 * u2 - 2)
    w2 = -3 * u3 + (4 * u2 + u)
    w3 = u3 - u2
    w = np.stack([w0, w1, w2, w3], axis=2)   # [V, d, i], raw 2x weights
    w[:, 2, :] *= np.float32(0.125)          # fold 1/8 into z
    wx, wy, wz = w[:, 0], w[:, 1], w[:, 2]
    W = (wz[:, :, None, None] * wx[:, None, :, None] *
         wy[:, None, None, :]).astype(np.float32)      # [V, k, i, j]
    return _f32_to_bf16_bits(W.reshape(len(vc), 64))


def _prepare(vert, vol):
    vc, fl = _clip_floor(vert)
    rows = _host_rows(fl)
    wbits = _host_weights(vc, fl)                # [V, 64] u16
    order = np.argsort(rows, kind="stable")
    Sfull = _build_shingle_u16(vol)

    cores = []
    nslab_needed = 0
    for c in range(NCORES):
        ids = order[c * VCORE:(c + 1) * VCORE]
        r = rows[ids]
        groups = []
        i = 0
        n = len(ids)
        while i < n:
            jmax = min(i + GROUP, n)
            j = int(np.searchsorted(r, r[i] + SPAN_CAP, side="right"))
            j = min(j, jmax)
            groups.append((i, j))
            i = j
        cores.append((ids, r, groups))
        nslab_needed = max(nslab_needed, len(groups))
    nslab = nslab_needed
    m = nslab * GCOLS
    srows = nslab * BSTRIDE

    in_maps = []
    perms = []
    for c in range(NCORES):
        ids, r, groups = cores[c]
        sh = np.zeros((srows, P), np.uint16)
        wtab = np.zeros((m * P, 64), np.uint16)
        idx16 = np.zeros((nslab, 16, GROUP // 16), np.int16)
        flat_pos = np.empty(len(ids), np.int64)
        for b in range(nslab):
            if b >= len(groups):
                continue
            i, j = groups[b]
            cnt = j - i
            r0 = int(r[i])
            r1 = int(r[j - 1])
            nrow = min(r1 - r0 + 4, srows - b * BSTRIDE)
            sh[b * BSTRIDE:b * BSTRIDE + nrow] = Sfull[r0:r0 + nrow]
            full_rel = np.zeros(GROUP, np.int16)
            full_rel[:cnt] = (r[i:j] - r0).astype(np.int16)
            idx16[b] = full_rel.reshape(GROUP // 16, 16).T
            jj = np.arange(cnt)
            part = jj % P
            col = b * GCOLS + jj // P
            flat_pos[i:i + cnt] = part * m + col
            wtab[part * m + col] = wbits[ids[i:j]]
        in_maps.append({
            "shingle": sh.view(ml_dtypes.bfloat16),
            "wtab": np.ascontiguousarray(
                wtab.reshape(P, m * 64)).view(ml_dtypes.bfloat16),
            "idx": np.ascontiguousarray(
                np.tile(idx16.transpose(1, 0, 2).reshape(16, -1), (8, 1))),
        })
        perms.append((ids, flat_pos))
    return nslab, in_maps, perms


def run_cores(vert, vol, trace=False, n_cores=NCORES, **kwargs):
    nslab, in_maps, perms = _prepare(vert, vol)
    nc = _get_program(nslab)
    res = run_bass_kernel_spmd(nc, in_maps, list(range(n_cores)),
                               trace=trace, **kwargs)
    m = nslab * GCOLS
    full = np.zeros((1, V, C), np.float32)
    for c in range(n_cores):
        out = np.asarray(res.results[c]["out"]).reshape(P * m, C)
        ids, flat_pos = perms[c]
        full[0, ids] = out[flat_pos]
    return full, res


def kernel(vert, vol):
    full, _ = run_cores(vert, vol, trace=False)
    return full


# revision 29
# speedup vs baseline: 1.0718x; 1.0208x over previous
"""Trainium2 Bass kernel for cubic (Keys) interpolation of vertices in a 3D volume.

bf16 shingle + slab-bucketed dma_gather + host-precomputed weight table
+ 2x-mode DVE multiply/tree pipeline.

Sharding: vertices are sorted by shingle row (host side) and split into 8
equal rank-ranges, one per NeuronCore. The volume is stored per core as a
bf16 shingle
    S[x, y, z, c, dx, dy] = vol[c, x+dx, y+dy, z]     (rows of 128 bf16)
with only the row-ranges that core's vertices touch, repacked into NSLAB
fixed-stride slabs so every gather index fits int16 (dma_gather's index
dtype). Rows (x,y,z)..(x,y,z+3) -- one 1KB run -- hold a vertex's whole
4x4x4x8 neighborhood in payload order [k(z):4, c:8, i(dx):4, j(dy):4];
channel sits mid-payload so the weight broadcast keeps innermost step=1 and
the large DVE ops run in 2x_1P bf16 mode.

Per core: NSLAB dma_gather calls (ONE SWDGE instruction each, spread over 4
SWDGE queues) fetch GROUP=896 vertex neighborhoods per call, landing index
j at (partition j%128, column j//128). Groups are padded to exactly 896
with fake vertices so the call structure is SPMD-uniform; fake outputs are
dropped at reassembly. The 64 Keys-kernel weights per vertex (wz*wx*wy,
separable outer product) are precomputed on the host in fp32, cast to bf16,
and uploaded as a [128, M*64] table -- they are O(V) metadata derived from
the same clip/floor as the gather indices. The device does the heavy part:
gather 1KB/vertex, multiply by the broadcast weights (DVE 2x), tree-reduce
k, i, j (2 ops each; the last j-add writes the compact f32 output), DMA out.
"""

import numpy as np
import ml_dtypes

import concourse.bass as bass
import concourse.tile as tile
from concourse import bacc, mybir
from concourse.bass_utils import run_bass_kernel_spmd

X, Y, Z, C = 112, 224, 160, 8
P = 128
NCORES = 8
V = 150000
VCORE = V // NCORES          # 18750
GROUP = 896                  # indices per dma_gather call (7 columns)
GCOLS = GROUP // P           # 7
SPAN_CAP = 32700             # max row span within one slab (int16 margin)
BSTRIDE = 32772              # rows per slab in the repacked shingle
ES = 512                     # elems gathered per index (4 rows x 128)
STEP = 128                   # elems per row
TILE_SLABS = 4               # slabs processed per DVE batch
GBUFS = 5
MAGIC = 12582912.0           # 1.5 * 2**23 fp32 round-to-int magic

BF16 = mybir.dt.bfloat16
F32 = mybir.dt.float32
I16 = mybir.dt.int16
ALU = mybir.AluOpType
ACT = mybir.ActivationFunctionType

_CACHE = {}


# --------------------------------------------------------------------------
# device program
# --------------------------------------------------------------------------

def _build_program(nslab):
    m = nslab * GCOLS  # total slot-columns per partition
    srows = nslab * BSTRIDE
    nc = bacc.Bacc("TRN2", target_bir_lowering=False, debug=False,
                   num_devices=NCORES, num_swdge_queues=4)
    s_in = nc.dram_tensor("shingle", [srows, P], BF16, kind="ExternalInput").ap()
    w_in = nc.dram_tensor("wtab", [P, m * 64], BF16, kind="ExternalInput").ap()
    idx_in = nc.dram_tensor("idx", [P, nslab * (GROUP // 16)], I16,
                            kind="ExternalInput").ap()
    out_ext = nc.dram_tensor("out", [P, m * C], F32, kind="ExternalOutput").ap()

    with tile.TileContext(nc) as tc:
        _emit(tc, nslab, out_ext, w_in, idx_in, s_in)
    nc.compile()
    return nc


def _emit(tc, nslab, out_ext, w_in, idx_in, s_in):
    nc = tc.nc
    vec = nc.vector
    m = nslab * GCOLS

    with (
        tc.tile_pool(name="keep", bufs=1) as keep,
        tc.tile_pool(name="gpool", bufs=GBUFS) as gpool,
        tc.tile_pool(name="opool", bufs=2) as opool,
    ):
        Wall = keep.tile([P, m * 64], BF16)
        idx = keep.tile([P, nslab * (GROUP // 16)], I16)

        nc.sync.dma_start(out=idx[:], in_=idx_in)
        nc.sync.dma_start(out=Wall[:], in_=w_in)

        # ramp-up batch schedule: small first batches so the DVE starts
        # as soon as the first slab lands, then steady TILE_SLABS batches
        sizes = [1, 2]
        left = nslab - 3
        while left > 0:
            t = min(TILE_SLABS, left)
            sizes.append(t)
            left -= t
        batches = []
        b0 = 0
        for t in sizes:
            batches.append((b0, b0 + t))
            b0 += t

        for (b0, b1) in batches:
            nb = b1 - b0
            ns = nb * GCOLS          # slots this batch
            s0 = b0 * GCOLS
            G = gpool.tile([P, TILE_SLABS * GCOLS * ES], BF16, tag="G")
            ot = opool.tile([P, TILE_SLABS * GCOLS * C], F32, tag="ot")

            # one dma_gather per slab: GROUP indices x 1KB runs
            for b in range(b0, b1):
                src_win = bass.AP(s_in.tensor, b * BSTRIDE * STEP,
                                  [[STEP, SPAN_CAP + 8], [1, ES]])
                gv = G[:, (b - b0) * GCOLS * ES:(b - b0 + 1) * GCOLS * ES] \
                    .rearrange("p (t e) -> p t e", e=ES)
                nc.gpsimd.dma_gather(
                    out_ap=gv, in_ap=src_win,
                    idxs_ap=idx[:, b * (GROUP // 16):(b + 1) * (GROUP // 16)],
                    num_idxs=GROUP, num_idxs_reg=GROUP,
                    elem_size=ES, elem_step=STEP, queue_num=b % 4)

            # G *= W  (payload [k, c, i, j]; c broadcast mid-dim -> 2x)
            Gv = G[:, :ns * ES].rearrange("p (s k c e) -> p s k c e",
                                          s=ns, k=4, c=8, e=16)
            vec.tensor_tensor(
                out=Gv, in0=Gv,
                in1=Wall[:, s0 * 64:(s0 + ns) * 64]
                    .rearrange("p (s k e) -> p s k e", s=ns, k=4)
                    .unsqueeze(3).to_broadcast([P, ns, 4, 8, 16]),
                op=ALU.mult)

            # k-tree (stride 128), 2 ops
            Gk = G[:, :ns * ES].rearrange("p (s k r) -> p s k r",
                                          s=ns, k=4, r=128)
            vec.tensor_tensor(out=Gk[:, :, 0:2], in0=Gk[:, :, 0:2],
                              in1=Gk[:, :, 2:4], op=ALU.add)
            vec.tensor_tensor(out=Gk[:, :, 0:1], in0=Gk[:, :, 0:1],
                              in1=Gk[:, :, 1:2], op=ALU.add)
            # i-tree, 2 ops
            Gi = G[:, :ns * ES].rearrange("p (s k c i j) -> p s k c i j",
                                          s=ns, k=4, c=8, i=4, j=4)[:, :, 0]
            vec.tensor_tensor(out=Gi[:, :, :, 0:2], in0=Gi[:, :, :, 0:2],
                              in1=Gi[:, :, :, 2:4], op=ALU.add)
            vec.tensor_tensor(out=Gi[:, :, :, 0:1], in0=Gi[:, :, :, 0:1],
                              in1=Gi[:, :, :, 1:2], op=ALU.add)
            # j-tree, 2 ops
            Gj = Gi[:, :, :, 0]
            vec.tensor_tensor(out=Gj[:, :, :, 0:2], in0=Gj[:, :, :, 0:2],
                              in1=Gj[:, :, :, 2:4], op=ALU.add)
            # final j-add writes the compact f32 output view directly
            vec.tensor_tensor(
                out=ot[:, :ns * C].rearrange("p (s c) -> p s c", c=C)
                    .unsqueeze(3),
                in0=Gj[:, :, :, 0:1], in1=Gj[:, :, :, 1:2], op=ALU.add)
            nc.sync.dma_start(out=out_ext[:, s0 * C:(s0 + ns) * C],
                              in_=ot[:, :ns * C])


def _get_program(nslab):
    key = ("nc", nslab)
    if key not in _CACHE:
        _CACHE[key] = _build_program(nslab)
    return _CACHE[key]


# --------------------------------------------------------------------------
# host-side preparation
# --------------------------------------------------------------------------

def _f32_to_bf16_bits(a):
    b = a.view(np.uint32)
    rounded = b + 0x7FFF + ((b >> 16) & 1)
    return (rounded >> 16).astype(np.uint16)


def _build_shingle_u16(vol):
    """S[x, y, z, c, dx, dy] = vol[c, x+dx, y+dy, z], flat [NROW, 128] u16."""
    v = np.ascontiguousarray(np.asarray(vol[0], dtype=np.float32))  # (C,X,Y,Z)
    vb = _f32_to_bf16_bits(v)
    vt = np.ascontiguousarray(vb.transpose(1, 2, 3, 0))             # (X,Y,Z,C)
    S = np.zeros((X, Y, Z, C, 4, 4), np.uint16)
    for dx in range(4):
        for dy in range(4):
            S[:X - dx, :Y - dy, :, :, dx, dy] = vt[dx:, dy:, :, :]
    return S.reshape(X * Y * Z, 128)


def _clip_floor(vert):
    """Exact replica of fp32 clip + magic-floor; returns (vc, fl)."""
    v = np.asarray(vert[0], dtype=np.float32)
    vc = np.empty_like(v)
    for d, dim in enumerate((X, Y, Z)):
        vc[:, d] = np.clip(v[:, d], np.float32(1.0 + 1e-5),
                           np.float32(dim - 2 - 1e-5))
    mg = np.float32(MAGIC)
    fl = ((vc - np.float32(0.5)) + mg) - mg
    return vc, fl


def _host_rows(fl):
    fli = fl.astype(np.int64)
    return ((fli[:, 0] - 1) * Y + (fli[:, 1] - 1)) * Z + (fli[:, 2] - 1)


def _host_weights(vc, fl):
    """Keys cubic weights -> [V, 4(k/z), 4(i/x), 4(j/y)] bf16 bits (u16)."""
    u = (vc - fl).astype(np.float32)
    u2 = u * u
    u3 = u2 * u
    w0 = -u3 + 2 * u2 - u
    w1 = 3 * u3 - (5 * u2 - 2)
    w2 = -3 * u3 + (4 * u2 + u)
    w3 = u3 - u2
    w = np.stack([w0, w1, w2, w3], axis=2)   # [V, d, i], raw 2x weights
    w[:, 2, :] *= np.float32(0.125)          # fold 1/8 into z
    wx, wy, wz = w[:, 0], w[:, 1], w[:, 2]
    W = (wz[:, :, None, None] * wx[:, None, :, None] *
         wy[:, None, None, :]).astype(np.float32)      # [V, k, i, j]
    return _f32_to_bf16_bits(W.reshape(len(vc), 64))


def _prepare(vert, vol):
    vc, fl = _clip_floor(vert)
    rows = _host_rows(fl)
    wbits = _host_weights(vc, fl)                # [V, 64] u16
    order = np.argsort(rows, kind="stable")
    Sfull = _build_shingle_u16(vol)

    cores = []
    nslab_needed = 0
    for c in range(NCORES):
        ids = order[c * VCORE:(c + 1) * VCORE]
        r = rows[ids]
        groups = []
        i = 0
        n = len(ids)
        while i < n:
            jmax = min(i + GROUP, n)
            j = int(np.searchsorted(r, r[i] + SPAN_CAP, side="right"))
            j = min(j, jmax)
            groups.append((i, j))
            i = j
        cores.append((ids, r, groups))
        nslab_needed = max(nslab_needed, len(groups))
    nslab = nslab_needed
    m = nslab * GCOLS
    srows = nslab * BSTRIDE

    in_maps = []
    perms = []
    for c in range(NCORES):
        ids, r, groups = cores[c]
        sh = np.zeros((srows, P), np.uint16)
        wtab = np.zeros((m * P, 64), np.uint16)
        idx16 = np.zeros((nslab, 16, GROUP // 16), np.int16)
        flat_pos = np.empty(len(ids), np.int64)
        for b in range(nslab):
            if b >= len(groups):
                continue
            i, j = groups[b]
            cnt = j - i
            r0 = int(r[i])
            r1 = int(r[j - 1])
            nrow = min(r1 - r0 + 4, srows - b * BSTRIDE)
            sh[b * BSTRIDE:b * BSTRIDE + nrow] = Sfull[r0:r0 + nrow]
            full_rel = np.zeros(GROUP, np.int16)
            full_rel[:cnt] = (r[i:j] - r0).astype(np.int16)
            idx16[b] = full_rel.reshape(GROUP // 16, 16).T
            jj = np.arange(cnt)
            part = jj % P
            col = b * GCOLS + jj // P
            flat_pos[i:i + cnt] = part * m + col
            wtab[part * m + col] = wbits[ids[i:j]]
        in_maps.append({
            "shingle": sh.view(ml_dtypes.bfloat16),
            "wtab": np.ascontiguousarray(
                wtab.reshape(P, m * 64)).view(ml_dtypes.bfloat16),
            "idx": np.ascontiguousarray(
                np.tile(idx16.transpose(1, 0, 2).reshape(16, -1), (8, 1))),
        })
        perms.append((ids, flat_pos))
    return nslab, in_maps, perms


def run_cores(vert, vol, trace=False, n_cores=NCORES, **kwargs):
    nslab, in_maps, perms = _prepare(vert, vol)
    nc = _get_program(nslab)
    res = run_bass_kernel_spmd(nc, in_maps, list(range(n_cores)),
                               trace=trace, **kwargs)
    m = nslab * GCOLS
    full = np.zeros((1, V, C), np.float32)
    for c in range(n_cores):
        out = np.asarray(res.results[c]["out"]).reshape(P * m, C)
        ids, flat_pos = perms[c]
        full[0, ids] = out[flat_pos]
    return full, res


def kernel(vert, vol):
    full, _ = run_cores(vert, vol, trace=False)
    return full
